# revision 7
# baseline (speedup 1.0000x reference)
"""Trainium2 Bass kernel for a 2-layer GCN + global mean pool + FC.

v4 strategy (8 NeuronCores, SPMD single NEFF):
  - Nodes (and in-edges) partitioned by dst across 8 cores.
  - Unified chunk plan for both layers: per dst block, one self chunk plus
    A-half and B-half gather chunks, where the halves split each owner's
    shard at local row 3200 (so both gather tables have < 32768 rows for
    int16 indices, and the A table is AllGathered mid-layer-1).
  - Norm-hot routing masks S (norm_e at the edge's dst column, 1/deg on
    the self diagonal) depend only on graph structure, are identical for
    both layers, and are host-built and streamed from HBM.
  - Layer 1 messages are host-expanded into a contiguous per-edge stream of
    raw bf16 x rows in chunk order -- no gathers, no Q7 work.
  - Layer-2 h1 rows live in 256B-padded AllGather outputs (fullA/fullB,
    gathered as padded shards -- no expand step).  The whole A-half of
    layer 2 (gather descriptor generation on GpSimd + matmuls) runs during
    layer 1's tail, staging partial aggregates in SBUF; after the final
    AllGather only the B-half remains.
  - agg accumulates transposed (aggT[64f,128d] += tile^T @ S_chunk): the
    epilogue is a direct matmul with W (bias via rank-1 ones x b matmul)
    plus one ACT tanh.  No transposes; DVE idle in the steady state (its
    2-port ops would lock GpSimd out of SBUF and stall descriptor
    generation).
"""

import numpy as np
import ml_dtypes

from concourse import bacc, bass, mybir, bass_utils
import concourse.tile as tile

N = 50000
E = 800000
F = 64
G = 128
OUT = 8
P = 128
C = 8
NSH = N // C          # 6250 nodes per core
NB = (NSH + P - 1) // P   # 49 dst blocks per core
SBLK = 4
NSET = (NB + SBLK - 1) // SBLK  # 13 sets
ABLOCKS = 25          # A half = owner-local rows [0, 3200)
SA_ROWS = ABLOCKS * P         # 3200
SB_ROWS = NSH - SA_ROWS       # 3050
NA_ROWS = C * SA_ROWS         # 25600 (A gather table)
NB_ROWS = C * SB_ROWS         # 24400 (B gather table)
F32 = mybir.dt.float32
BF16 = mybir.dt.bfloat16
I16 = mybir.dt.int16
BF = ml_dtypes.bfloat16


def _pieces(n, k):
    out = []
    step = (n + k - 1) // k
    for c0 in range(0, n, step):
        out.append((c0, min(c0 + step, n)))
    return out


def _set_blocks(s):
    return list(range(s * SBLK, min((s + 1) * SBLK, NB)))


def _preprocess(x, src, dst, batch):
    """Host-side planning: index work + layout transforms of the inputs."""
    src = np.asarray(src).astype(np.int64)
    dst = np.asarray(dst).astype(np.int64)
    batch = np.asarray(batch).astype(np.int64)
    xb = np.asarray(x, np.float32).astype(BF)

    deg = np.bincount(dst, minlength=N).astype(np.float64) + 1.0
    dinv = 1.0 / np.sqrt(deg)

    owner = src // NSH
    local = src % NSH
    isA = local < SA_ROWS
    gidx = np.where(isA, owner * SA_ROWS + local,
                    owner * SB_ROWS + (local - SA_ROWS))

    # per-core, per-(block, half) edge lists sorted by gather index
    core_e = []     # [c][b] -> (srcA, giA, dlA, srcB, giB, dlB)
    for c in range(C):
        lo = c * NSH
        m = (dst >= lo) & (dst < lo + NSH)
        es, gi, ia, ed = src[m], gidx[m], isA[m], dst[m] - lo
        blk = ed >> 7
        dl = ed & 127
        order = np.lexsort((gi, blk))
        es, gi, ia, dl, blk = (a[order] for a in (es, gi, ia, dl, blk))
        bounds = np.searchsorted(blk, np.arange(NB + 1))
        per_b = []
        for b in range(NB):
            g0, g1 = bounds[b], bounds[b + 1]
            e, g, i2, d = es[g0:g1], gi[g0:g1], ia[g0:g1], dl[g0:g1]
            per_b.append((e[i2], g[i2], d[i2], e[~i2], g[~i2], d[~i2]))
        core_e.append(per_b)

    cntA = np.zeros((C, NB), np.int64)
    cntB = np.zeros((C, NB), np.int64)
    for c in range(C):
        for b in range(NB):
            t = core_e[c][b]
            cntA[c, b] = len(t[0])
            cntB[c, b] = len(t[3])
    nchA = np.maximum(np.ceil(cntA.max(axis=0) / P).astype(np.int64), 1)
    nchB = np.maximum(np.ceil(cntB.max(axis=0) / P).astype(np.int64), 1)

    n_set = np.array([sum(1 + nchA[b] + nchB[b] for b in _set_blocks(s))
                      for s in range(NSET)])
    NCHT = int(n_set.sum())
    nA_set = np.array([sum(nchA[b] for b in _set_blocks(s)) for s in range(NSET)])
    nB_set = np.array([sum(nchB[b] for b in _set_blocks(s)) for s in range(NSET)])
    icolsA = int(nA_set.sum()) * (P // 16)
    icolsB = int(nB_set.sum()) * (P // 16)

    # per-set column layout: per block: [self][A chunks][B chunks]
    # runs[s][b] = (self_col, a0, b0) as set-relative chunk columns
    runs = []
    sched = []
    for s in range(NSET):
        rr = {}
        lst = []
        k = 0
        ao = bo = 0
        for b in _set_blocks(s):
            rr[b] = (k, k + 1, k + 1 + int(nchA[b]))
            lst.append((b, 0, 0))
            k += 1
            for i in range(int(nchA[b])):
                lst.append((b, 1, ao)); ao += 1; k += 1
            for i in range(int(nchB[b])):
                lst.append((b, 2, bo)); bo += 1; k += 1
        runs.append(rr)
        sched.append(lst)

    plan = dict(nchA=nchA, nchB=nchB, n_set=n_set, nA_set=nA_set,
                nB_set=nB_set, NCHT=NCHT, icolsA=icolsA, icolsB=icolsB,
                sched=sched, runs=runs)

    per_core = []
    for c in range(C):
        xs = np.zeros((P, NCHT, F), BF)
        Sm = np.zeros((P, NCHT, P), BF)
        idxA_parts, idxB_parts = [], []
        ch = 0
        for s in range(NSET):
            for b in _set_blocks(s):
                _, giA, dA, _, giB, dB = core_e[c][b]
                nr = min(P, NSH - b * P)
                own = c * NSH + b * P + np.arange(nr)
                xs[:nr, ch, :] = xb[own]
                Sm[np.arange(nr), ch, np.arange(nr)] = (1.0 / deg[own]).astype(BF)
                ch += 1
                for half, (gis, dls_all, parts) in enumerate(
                        ((giA, dA, idxA_parts), (giB, dB, idxB_parts))):
                    nchs = int(nchA[b]) if half == 0 else int(nchB[b])
                    for i in range(nchs):
                        rows = gis[i * P:(i + 1) * P]
                        dls = dls_all[i * P:(i + 1) * P]
                        nr = len(rows)
                        gi2 = np.zeros(P, np.int64)
                        gi2[:nr] = rows
                        parts.append(gi2)
                        if nr:
                            gl = (rows // (SA_ROWS if half == 0 else SB_ROWS)) * NSH \
                                + rows % (SA_ROWS if half == 0 else SB_ROWS) \
                                + (0 if half == 0 else SA_ROWS)
                            xs[:nr, ch, :] = xb[gl]
                            nrm = (dinv[gl] * dinv[c * NSH + b * P + dls]).astype(BF)
                            Sm[np.arange(nr), ch, dls] = nrm
                        ch += 1
        assert ch == NCHT

        def mk_idx(parts):
            if not parts:
                return np.zeros((P, 8), np.int16)
            stk = np.concatenate(parts).astype(np.int16)
            return np.tile(stk.reshape(-1, 16).T, (8, 1))

        Sp = np.zeros((P, NB, G), BF)
        own = np.arange(NSH)
        Sp[own & 127, own >> 7, batch[c * NSH + own]] = 1.0
        per_core.append(dict(
            xs=np.ascontiguousarray(xs.reshape(P, NCHT * F)),
            Sm=np.ascontiguousarray(Sm.reshape(P, NCHT * P)),
            idxA=mk_idx(idxA_parts), idxB=mk_idx(idxB_parts),
            Sp=np.ascontiguousarray(Sp.reshape(P, NB * G)),
        ))

    cnt = np.bincount(batch, minlength=G).astype(np.float32)
    invc = (1.0 / np.maximum(cnt, 1.0)).astype(np.float32)
    return plan, per_core, invc


def _build(plan):
    nchA, nchB = plan["nchA"], plan["nchB"]
    n_set, nA_set, nB_set = plan["n_set"], plan["nA_set"], plan["nB_set"]
    NCHT = plan["NCHT"]
    icolsA, icolsB = plan["icolsA"], plan["icolsB"]
    sched, runs = plan["sched"], plan["runs"]
    set_off = np.concatenate(([0], np.cumsum(n_set)))

    nc = bacc.Bacc("TRN2", target_bir_lowering=False, debug=False,
                   num_devices=C, num_swdge_queues=4)

    xs_in = nc.dram_tensor("xs", [P, NCHT * F], BF16, kind="ExternalInput")
    Sm_in = nc.dram_tensor("Sm", [P, NCHT * P], BF16, kind="ExternalInput")
    idxA_in = nc.dram_tensor("idxA", [P, max(icolsA, 8)], I16, kind="ExternalInput")
    idxB_in = nc.dram_tensor("idxB", [P, max(icolsB, 8)], I16, kind="ExternalInput")
    Sp_in = nc.dram_tensor("Sp", [P, NB * G], BF16, kind="ExternalInput")
    id64_in = nc.dram_tensor("id64", [F, F], BF16, kind="ExternalInput")
    W1_in = nc.dram_tensor("W1", [F, F], BF16, kind="ExternalInput")
    b1_in = nc.dram_tensor("b1r", [1, F], BF16, kind="ExternalInput")
    W2_in = nc.dram_tensor("W2", [F, F], BF16, kind="ExternalInput")
    b2_in = nc.dram_tensor("b2r", [1, F], BF16, kind="ExternalInput")
    Wfc_in = nc.dram_tensor("Wfc", [F, OUT], F32, kind="ExternalInput")
    bfc_in = nc.dram_tensor("bfcb", [P, OUT], F32, kind="ExternalInput")
    invc_in = nc.dram_tensor("invc", [F, G], F32, kind="ExternalInput")
    out = nc.dram_tensor("out", [G, OUT], F32, kind="ExternalOutput")

    gq = [0]

    with tile.TileContext(nc) as tc:
        with (
            tc.tile_pool(name="const", bufs=1) as cp,
            tc.tile_pool(name="stream", bufs=2) as stp,
            tc.tile_pool(name="smask", bufs=2) as smp,
            tc.tile_pool(name="smA", bufs=2) as smA,
            tc.tile_pool(name="smB", bufs=2) as smB,
            tc.tile_pool(name="gpool", bufs=3) as gp,
            tc.tile_pool(name="epool", bufs=4) as ep,
            tc.tile_pool(name="psA", bufs=3, space="PSUM") as psA,
            tc.tile_pool(name="psH", bufs=2, space="PSUM") as psH,
            tc.tile_pool(name="psP", bufs=1, space="PSUM") as psP,
            tc.tile_pool(name="dram", bufs=1, space="DRAM") as dram,
        ):
            # ---- constants ----
            idxA_sb = cp.tile([P, max(icolsA, 8)], I16, tag="idxA")
            nc.scalar.dma_start(idxA_sb[:], idxA_in[:])
            idxB_sb = cp.tile([P, max(icolsB, 8)], I16, tag="idxB")
            nc.scalar.dma_start(idxB_sb[:], idxB_in[:])
            Sp_sb = cp.tile([P, NB, G], BF16, tag="Sp")
            nc.scalar.dma_start(Sp_sb[:], Sp_in[:])
            id64_sb = cp.tile([F, F], BF16, tag="id64")
            nc.sync.dma_start(id64_sb[:], id64_in[:])
            W1_sb = cp.tile([F, F], BF16, tag="W1")
            nc.sync.dma_start(W1_sb[:], W1_in[:])
            b1_sb = cp.tile([1, F], BF16, tag="b1")
            nc.sync.dma_start(b1_sb[:], b1_in[:])
            W2_sb = cp.tile([F, F], BF16, tag="W2")
            nc.sync.dma_start(W2_sb[:], W2_in[:])
            b2_sb = cp.tile([1, F], BF16, tag="b2")
            nc.sync.dma_start(b2_sb[:], b2_in[:])
            Wfc_sb = cp.tile([F, OUT], F32, tag="Wfc")
            nc.sync.dma_start(Wfc_sb[:], Wfc_in[:])
            bfc_sb = cp.tile([P, OUT], F32, tag="bfc")
            nc.sync.dma_start(bfc_sb[:], bfc_in[:])
            invc_sb = cp.tile([F, G], F32, tag="invc")
            nc.sync.dma_start(invc_sb[:], invc_in[:])
            ones_sb = cp.tile([1, P], BF16, tag="ones")
            nc.vector.memset(ones_sb[:], 1.0)

            own_sb = cp.tile([P, NB, F], BF16, tag="own")
            aggA_sb = cp.tile([F, NB, P], BF16, tag="aggA")

            shardA = dram.tile([SA_ROWS, 2 * F], BF16)
            shardB = dram.tile([SB_ROWS, 2 * F], BF16)
            fullA = dram.tile([NA_ROWS, 2 * F], BF16, addr_space="Shared")
            fullB = dram.tile([NB_ROWS, 2 * F], BF16, addr_space="Shared")
            pool_in = dram.tile([F, G], F32)
            pool_out = dram.tile([F, G], F32, addr_space="Shared")

            pool_ps = psP.tile([F, G], F32, tag="pool")

            def epilogue(b, aggT, W_sb, brow_sb, layer):
                agg_sb = ep.tile([F, P], BF16, tag="agg_sb")
                nc.scalar.copy(agg_sb[:], aggT[:])
                h_ps = psH.tile([P, F], F32, tag="h")
                nc.tensor.matmul(h_ps[:], lhsT=agg_sb[:], rhs=W_sb[:],
                                 start=True, stop=False)
                nc.tensor.matmul(h_ps[:], lhsT=ones_sb[:], rhs=brow_sb[:],
                                 start=False, stop=True)
                if layer == 1:
                    nc.scalar.activation(own_sb[:, b, :], h_ps[:],
                                         mybir.ActivationFunctionType.Tanh)
                    if b < ABLOCKS:
                        r0 = b * P
                        nc.sync.dma_start(shardA[r0:r0 + P, 0:F],
                                          own_sb[:, b, :])
                    else:
                        r0 = (b - ABLOCKS) * P
                        rows = min(P, SB_ROWS - r0)
                        nc.sync.dma_start(shardB[r0:r0 + rows, 0:F],
                                          own_sb[:rows, b, :])
                else:
                    h2t = ep.tile([P, F], BF16, tag="h2t")
                    nc.scalar.activation(h2t[:], h_ps[:],
                                         mybir.ActivationFunctionType.Tanh)
                    nc.tensor.matmul(pool_ps[:], lhsT=h2t[:],
                                     rhs=Sp_sb[:, b, :],
                                     start=(b == 0), stop=(b == NB - 1),
                                     skip_group_check=True)

            # =================== Layer 1 (streamed) ===================
            for s in range(NSET):
                ns = int(n_set[s])
                c0 = int(set_off[s])
                st = stp.tile([P, ns * F], BF16, tag="st")
                nc.scalar.dma_start(st[:], xs_in[:, c0 * F:(c0 + ns) * F])
                S_t = smp.tile([P, ns * P], BF16, tag="Sm")
                nc.sync.dma_start(S_t[:], Sm_in[:, c0 * P:(c0 + ns) * P])
                mms = sched[s]
                for b in _set_blocks(s):
                    kis = [k for k, (bb, kind, col) in enumerate(mms) if bb == b]
                    aggT = psA.tile([F, P], F32, tag="aggT")
                    for j, k in enumerate(kis):
                        nc.tensor.matmul(
                            aggT[:],
                            lhsT=st[:, k * F:(k + 1) * F],
                            rhs=S_t[:, k * P:(k + 1) * P],
                            start=(j == 0), stop=(j == len(kis) - 1),
                        )
                    epilogue(b, aggT, W1_sb, b1_sb, 1)
                    if b == ABLOCKS - 1:
                        nc.gpsimd.collective_compute(
                            "AllGather", mybir.AluOpType.bypass,
                            ins=[shardA.opt()], outs=[fullA.opt()],
                            replica_groups=[list(range(C))],
                        )

            # ============ Layer 2, A half (overlaps L1 tail) ============
            acol = 0
            for s in range(NSET):
                nAs = int(nA_set[s])
                c0 = int(set_off[s])
                gtA = gp.tile([P, nAs, 2 * F], BF16, tag="gtA")
                for (p0, p1) in _pieces(nAs, 2):
                    q = gq[0] % 4
                    gq[0] += 1
                    nc.gpsimd.dma_gather(
                        gtA[:, p0:p1, :], fullA[:],
                        idxA_sb[:, acol + p0 * 8:acol + p1 * 8],
                        (p1 - p0) * P, (p1 - p0) * P, 2 * F,
                        single_packet=False, queue_num=q,
                    )
                SA_t = smA.tile([P, nAs * P], BF16, tag="SA")
                ao = 0
                for b in _set_blocks(s):
                    scol, a0, b0 = runs[s][b]
                    na = int(nchA[b])
                    nc.sync.dma_start(
                        SA_t[:, ao * P:(ao + na) * P],
                        Sm_in[:, (c0 + a0) * P:(c0 + a0 + na) * P])
                    aggT = psA.tile([F, P], F32, tag="aggT")
                    for j in range(na):
                        nc.tensor.matmul(
                            aggT[:], lhsT=gtA[:, ao + j, 0:F],
                            rhs=SA_t[:, (ao + j) * P:(ao + j + 1) * P],
                            start=(j == 0), stop=(j == na - 1),
                        )
                    nc.scalar.copy(aggA_sb[:, b, :], aggT[:])
                    ao += na
                acol += nAs * 8
                if s == 6:
                    nc.gpsimd.collective_compute(
                        "AllGather", mybir.AluOpType.bypass,
                        ins=[shardB.opt()], outs=[fullB.opt()],
                        replica_groups=[list(range(C))],
                    )

            # =================== Layer 2, B half ===================
            bcol = 0
            for s in range(NSET):
                nBs = int(nB_set[s])
                c0 = int(set_off[s])
                gtB = gp.tile([P, nBs, 2 * F], BF16, tag="gtB")
                for (p0, p1) in _pieces(nBs, 2):
                    q = gq[0] % 4
                    gq[0] += 1
                    nc.gpsimd.dma_gather(
                        gtB[:, p0:p1, :], fullB[:],
                        idxB_sb[:, bcol + p0 * 8:bcol + p1 * 8],
                        (p1 - p0) * P, (p1 - p0) * P, 2 * F,
                        single_packet=False, queue_num=q,
                    )
                SB_t = smB.tile([P, (nBs + SBLK) * P], BF16, tag="SB")
                bo = 0
                srow = 0
                srows = {}
                for b in _set_blocks(s):
                    scol, a0, b0 = runs[s][b]
                    nb = int(nchB[b])
                    # self col + B run
                    nc.sync.dma_start(
                        SB_t[:, srow * P:(srow + 1) * P],
                        Sm_in[:, (c0 + scol) * P:(c0 + scol + 1) * P])
                    nc.sync.dma_start(
                        SB_t[:, (srow + 1) * P:(srow + 1 + nb) * P],
                        Sm_in[:, (c0 + b0) * P:(c0 + b0 + nb) * P])
                    srows[b] = srow
                    srow += 1 + nb
                for b in _set_blocks(s):
                    nb = int(nchB[b])
                    sr = srows[b]
                    aggT = psA.tile([F, P], F32, tag="aggT")
                    nc.tensor.matmul(aggT[:], lhsT=id64_sb[:],
                                     rhs=aggA_sb[:, b, :],
                                     start=True, stop=False)
                    nc.tensor.matmul(aggT[:], lhsT=own_sb[:, b, :],
                                     rhs=SB_t[:, sr * P:(sr + 1) * P],
                                     start=False, stop=False)
                    for j in range(nb):
                        nc.tensor.matmul(
                            aggT[:], lhsT=gtB[:, bo + j, 0:F],
                            rhs=SB_t[:, (sr + 1 + j) * P:(sr + 2 + j) * P],
                            start=False, stop=(j == nb - 1),
                        )
                    epilogue(b, aggT, W2_sb, b2_sb, 2)
                    bo += nb
                bcol += nBs * 8

            # ---- pooled tail ----
            poolT = ep.tile([F, G], F32, tag="poolT")
            nc.vector.tensor_copy(poolT[:], pool_ps[:])
            nc.sync.dma_start(pool_in[:], poolT[:])
            nc.gpsimd.collective_compute(
                "AllReduce", mybir.AluOpType.add,
                ins=[pool_in.opt()], outs=[pool_out.opt()],
                replica_groups=[list(range(C))],
            )
            poolR = ep.tile([F, G], F32, tag="poolR")
            nc.sync.dma_start(poolR[:], pool_out[:])
            nc.vector.tensor_mul(poolR[:], poolR[:], invc_sb[:])
            fc_ps = psP.tile([G, OUT], F32, tag="fc")
            nc.tensor.matmul(fc_ps[:], lhsT=poolR[:], rhs=Wfc_sb[:],
                             start=True, stop=True)
            out_sb = ep.tile([G, OUT], F32, tag="out_sb")
            nc.vector.tensor_add(out_sb[:], fc_ps[:], bfc_sb[:])
            nc.sync.dma_start(out[:], out_sb[:])

    nc.compile()
    return nc


def _in_maps(plan, per_core, invc, W1, b1, W2, b2, Wfc, bfc):
    com = dict(
        id64=np.eye(F, dtype=np.float32).astype(BF),
        W1=np.asarray(W1, np.float32).astype(BF),
        b1r=np.asarray(b1, np.float32).reshape(1, F).astype(BF),
        W2=np.asarray(W2, np.float32).astype(BF),
        b2r=np.asarray(b2, np.float32).reshape(1, F).astype(BF),
        Wfc=np.ascontiguousarray(np.asarray(Wfc, np.float32)),
        bfcb=np.tile(np.asarray(bfc, np.float32), (P, 1)),
        invc=np.tile(invc, (F, 1)),
    )
    maps = []
    for c in range(C):
        m = dict(com)
        m.update(per_core[c])
        maps.append({k: np.ascontiguousarray(v) for k, v in m.items()})
    return maps


_RUN_KWARGS = {}


def kernel(x, src, dst, batch, W1, b1, W2, b2, Wfc, bfc):
    plan, per_core, invc = _preprocess(x, src, dst, batch)
    nc = _build(plan)
    maps = _in_maps(plan, per_core, invc, W1, b1, W2, b2, Wfc, bfc)
    res = bass_utils.run_bass_kernel_spmd(
        nc, maps, core_ids=list(range(C)), **_RUN_KWARGS
    )
    kernel.last_results = res
    return np.asarray(res.results[0]["out"], np.float32)


# revision 8
# speedup vs baseline: 1.0099x; 1.0099x over previous
"""Trainium2 Bass kernel for a 2-layer GCN + global mean pool + FC.

v4 strategy (8 NeuronCores, SPMD single NEFF):
  - Nodes (and in-edges) partitioned by dst across 8 cores.
  - Unified chunk plan for both layers: per dst block, one self chunk plus
    A-half and B-half gather chunks, where the halves split each owner's
    shard at local row 3200 (so both gather tables have < 32768 rows for
    int16 indices, and the A table is AllGathered mid-layer-1).
  - Norm-hot routing masks S (norm_e at the edge's dst column, 1/deg on
    the self diagonal) depend only on graph structure, are identical for
    both layers, and are host-built and streamed from HBM.
  - Layer 1 messages are host-expanded into a contiguous per-edge stream of
    raw bf16 x rows in chunk order -- no gathers, no Q7 work.
  - Layer-2 h1 rows live in 256B-padded AllGather outputs (fullA/fullB,
    gathered as padded shards -- no expand step).  The whole A-half of
    layer 2 (gather descriptor generation on GpSimd + matmuls) runs during
    layer 1's tail, staging partial aggregates in SBUF; after the final
    AllGather only the B-half remains.
  - agg accumulates transposed (aggT[64f,128d] += tile^T @ S_chunk): the
    epilogue is a direct matmul with W (bias via rank-1 ones x b matmul)
    plus one ACT tanh.  No transposes; DVE idle in the steady state (its
    2-port ops would lock GpSimd out of SBUF and stall descriptor
    generation).
"""

import numpy as np
import ml_dtypes

from concourse import bacc, bass, mybir, bass_utils
import concourse.tile as tile

N = 50000
E = 800000
F = 64
G = 128
OUT = 8
P = 128
C = 8
NSH = N // C          # 6250 nodes per core
NB = (NSH + P - 1) // P   # 49 dst blocks per core
SBLK = 4
NSET = (NB + SBLK - 1) // SBLK  # 13 sets
ABLOCKS = 25          # A half = owner-local rows [0, 3200)
SA_ROWS = ABLOCKS * P         # 3200
SB_ROWS = NSH - SA_ROWS       # 3050
NA_ROWS = C * SA_ROWS         # 25600 (A gather table)
NB_ROWS = C * SB_ROWS         # 24400 (B gather table)
F32 = mybir.dt.float32
BF16 = mybir.dt.bfloat16
I16 = mybir.dt.int16
BF = ml_dtypes.bfloat16


def _pieces(n, k):
    out = []
    step = (n + k - 1) // k
    for c0 in range(0, n, step):
        out.append((c0, min(c0 + step, n)))
    return out


def _set_blocks(s):
    return list(range(s * SBLK, min((s + 1) * SBLK, NB)))


def _preprocess(x, src, dst, batch):
    """Host-side planning: index work + layout transforms of the inputs."""
    src = np.asarray(src).astype(np.int64)
    dst = np.asarray(dst).astype(np.int64)
    batch = np.asarray(batch).astype(np.int64)
    xb = np.asarray(x, np.float32).astype(BF)

    deg = np.bincount(dst, minlength=N).astype(np.float64) + 1.0
    dinv = 1.0 / np.sqrt(deg)

    owner = src // NSH
    local = src % NSH
    isA = local < SA_ROWS
    gidx = np.where(isA, owner * SA_ROWS + local,
                    owner * SB_ROWS + (local - SA_ROWS))

    # per-core, per-(block, half) edge lists sorted by gather index
    core_e = []     # [c][b] -> (srcA, giA, dlA, srcB, giB, dlB)
    for c in range(C):
        lo = c * NSH
        m = (dst >= lo) & (dst < lo + NSH)
        es, gi, ia, ed = src[m], gidx[m], isA[m], dst[m] - lo
        blk = ed >> 7
        dl = ed & 127
        order = np.lexsort((gi, blk))
        es, gi, ia, dl, blk = (a[order] for a in (es, gi, ia, dl, blk))
        bounds = np.searchsorted(blk, np.arange(NB + 1))
        per_b = []
        for b in range(NB):
            g0, g1 = bounds[b], bounds[b + 1]
            e, g, i2, d = es[g0:g1], gi[g0:g1], ia[g0:g1], dl[g0:g1]
            per_b.append((e[i2], g[i2], d[i2], e[~i2], g[~i2], d[~i2]))
        core_e.append(per_b)

    cntA = np.zeros((C, NB), np.int64)
    cntB = np.zeros((C, NB), np.int64)
    for c in range(C):
        for b in range(NB):
            t = core_e[c][b]
            cntA[c, b] = len(t[0])
            cntB[c, b] = len(t[3])
    nchA = np.maximum(np.ceil(cntA.max(axis=0) / P).astype(np.int64), 1)
    nchB = np.maximum(np.ceil(cntB.max(axis=0) / P).astype(np.int64), 1)

    n_set = np.array([sum(1 + nchA[b] + nchB[b] for b in _set_blocks(s))
                      for s in range(NSET)])
    NCHT = int(n_set.sum())
    nA_set = np.array([sum(nchA[b] for b in _set_blocks(s)) for s in range(NSET)])
    nB_set = np.array([sum(nchB[b] for b in _set_blocks(s)) for s in range(NSET)])
    icolsA = int(nA_set.sum()) * (P // 16)
    icolsB = int(nB_set.sum()) * (P // 16)

    # per-set column layout: per block: [self][A chunks][B chunks]
    # runs[s][b] = (self_col, a0, b0) as set-relative chunk columns
    runs = []
    sched = []
    for s in range(NSET):
        rr = {}
        lst = []
        k = 0
        ao = bo = 0
        for b in _set_blocks(s):
            rr[b] = (k, k + 1, k + 1 + int(nchA[b]))
            lst.append((b, 0, 0))
            k += 1
            for i in range(int(nchA[b])):
                lst.append((b, 1, ao)); ao += 1; k += 1
            for i in range(int(nchB[b])):
                lst.append((b, 2, bo)); bo += 1; k += 1
        runs.append(rr)
        sched.append(lst)

    plan = dict(nchA=nchA, nchB=nchB, n_set=n_set, nA_set=nA_set,
                nB_set=nB_set, NCHT=NCHT, icolsA=icolsA, icolsB=icolsB,
                sched=sched, runs=runs)

    per_core = []
    for c in range(C):
        xs = np.zeros((P, NCHT, F), BF)
        Sm = np.zeros((P, NCHT, P), BF)
        idxA_parts, idxB_parts = [], []
        ch = 0
        for s in range(NSET):
            for b in _set_blocks(s):
                _, giA, dA, _, giB, dB = core_e[c][b]
                nr = min(P, NSH - b * P)
                own = c * NSH + b * P + np.arange(nr)
                xs[:nr, ch, :] = xb[own]
                Sm[np.arange(nr), ch, np.arange(nr)] = (1.0 / deg[own]).astype(BF)
                ch += 1
                for half, (gis, dls_all, parts) in enumerate(
                        ((giA, dA, idxA_parts), (giB, dB, idxB_parts))):
                    nchs = int(nchA[b]) if half == 0 else int(nchB[b])
                    for i in range(nchs):
                        rows = gis[i * P:(i + 1) * P]
                        dls = dls_all[i * P:(i + 1) * P]
                        nr = len(rows)
                        gi2 = np.zeros(P, np.int64)
                        gi2[:nr] = rows
                        parts.append(gi2)
                        if nr:
                            gl = (rows // (SA_ROWS if half == 0 else SB_ROWS)) * NSH \
                                + rows % (SA_ROWS if half == 0 else SB_ROWS) \
                                + (0 if half == 0 else SA_ROWS)
                            xs[:nr, ch, :] = xb[gl]
                            nrm = (dinv[gl] * dinv[c * NSH + b * P + dls]).astype(BF)
                            Sm[np.arange(nr), ch, dls] = nrm
                        ch += 1
        assert ch == NCHT

        def mk_idx(parts):
            if not parts:
                return np.zeros((P, 8), np.int16)
            stk = np.concatenate(parts).astype(np.int16)
            return np.tile(stk.reshape(-1, 16).T, (8, 1))

        Sp = np.zeros((P, NB, G), BF)
        own = np.arange(NSH)
        Sp[own & 127, own >> 7, batch[c * NSH + own]] = 1.0
        per_core.append(dict(
            xs=np.ascontiguousarray(xs.reshape(P, NCHT * F)),
            Sm=np.ascontiguousarray(Sm.reshape(P, NCHT * P)),
            idxA=mk_idx(idxA_parts), idxB=mk_idx(idxB_parts),
            Sp=np.ascontiguousarray(Sp.reshape(P, NB * G)),
        ))

    cnt = np.bincount(batch, minlength=G).astype(np.float32)
    invc = (1.0 / np.maximum(cnt, 1.0)).astype(np.float32)
    return plan, per_core, invc


def _build(plan):
    nchA, nchB = plan["nchA"], plan["nchB"]
    n_set, nA_set, nB_set = plan["n_set"], plan["nA_set"], plan["nB_set"]
    NCHT = plan["NCHT"]
    icolsA, icolsB = plan["icolsA"], plan["icolsB"]
    sched, runs = plan["sched"], plan["runs"]
    set_off = np.concatenate(([0], np.cumsum(n_set)))

    nc = bacc.Bacc("TRN2", target_bir_lowering=False, debug=False,
                   num_devices=C, num_swdge_queues=4)

    xs_in = nc.dram_tensor("xs", [P, NCHT * F], BF16, kind="ExternalInput")
    Sm_in = nc.dram_tensor("Sm", [P, NCHT * P], BF16, kind="ExternalInput")
    idxA_in = nc.dram_tensor("idxA", [P, max(icolsA, 8)], I16, kind="ExternalInput")
    idxB_in = nc.dram_tensor("idxB", [P, max(icolsB, 8)], I16, kind="ExternalInput")
    Sp_in = nc.dram_tensor("Sp", [P, NB * G], BF16, kind="ExternalInput")
    id64_in = nc.dram_tensor("id64", [F, F], BF16, kind="ExternalInput")
    W1_in = nc.dram_tensor("W1", [F, F], BF16, kind="ExternalInput")
    b1_in = nc.dram_tensor("b1r", [1, F], BF16, kind="ExternalInput")
    W2_in = nc.dram_tensor("W2", [F, F], BF16, kind="ExternalInput")
    b2_in = nc.dram_tensor("b2r", [1, F], BF16, kind="ExternalInput")
    Wfc_in = nc.dram_tensor("Wfc", [F, OUT], F32, kind="ExternalInput")
    bfc_in = nc.dram_tensor("bfcb", [P, OUT], F32, kind="ExternalInput")
    invc_in = nc.dram_tensor("invc", [F, G], F32, kind="ExternalInput")
    out = nc.dram_tensor("out", [G, OUT], F32, kind="ExternalOutput")

    gq = [0]

    with tile.TileContext(nc) as tc:
        with (
            tc.tile_pool(name="const", bufs=1) as cp,
            tc.tile_pool(name="stream", bufs=2) as stp,
            tc.tile_pool(name="smask", bufs=2) as smp,
            tc.tile_pool(name="smA", bufs=2) as smA,
            tc.tile_pool(name="smB", bufs=2) as smB,
            tc.tile_pool(name="gpool", bufs=3) as gp,
            tc.tile_pool(name="epool", bufs=4) as ep,
            tc.tile_pool(name="psA", bufs=3, space="PSUM") as psA,
            tc.tile_pool(name="psH", bufs=2, space="PSUM") as psH,
            tc.tile_pool(name="psP", bufs=1, space="PSUM") as psP,
            tc.tile_pool(name="dram", bufs=1, space="DRAM") as dram,
        ):
            # ---- constants ----
            idxA_sb = cp.tile([P, max(icolsA, 8)], I16, tag="idxA")
            nc.scalar.dma_start(idxA_sb[:], idxA_in[:])
            idxB_sb = cp.tile([P, max(icolsB, 8)], I16, tag="idxB")
            nc.scalar.dma_start(idxB_sb[:], idxB_in[:])
            Sp_sb = cp.tile([P, NB, G], BF16, tag="Sp")
            nc.scalar.dma_start(Sp_sb[:], Sp_in[:])
            id64_sb = cp.tile([F, F], BF16, tag="id64")
            nc.sync.dma_start(id64_sb[:], id64_in[:])
            W1_sb = cp.tile([F, F], BF16, tag="W1")
            nc.sync.dma_start(W1_sb[:], W1_in[:])
            b1_sb = cp.tile([1, F], BF16, tag="b1")
            nc.sync.dma_start(b1_sb[:], b1_in[:])
            W2_sb = cp.tile([F, F], BF16, tag="W2")
            nc.sync.dma_start(W2_sb[:], W2_in[:])
            b2_sb = cp.tile([1, F], BF16, tag="b2")
            nc.sync.dma_start(b2_sb[:], b2_in[:])
            Wfc_sb = cp.tile([F, OUT], F32, tag="Wfc")
            nc.sync.dma_start(Wfc_sb[:], Wfc_in[:])
            bfc_sb = cp.tile([P, OUT], F32, tag="bfc")
            nc.sync.dma_start(bfc_sb[:], bfc_in[:])
            invc_sb = cp.tile([F, G], F32, tag="invc")
            nc.sync.dma_start(invc_sb[:], invc_in[:])
            ones_sb = cp.tile([1, P], BF16, tag="ones")
            nc.vector.memset(ones_sb[:], 1.0)

            own_sb = cp.tile([P, NB, F], BF16, tag="own")
            aggA_sb = cp.tile([F, NB, P], BF16, tag="aggA")

            shardA = dram.tile([SA_ROWS, 2 * F], BF16)
            shardB = dram.tile([SB_ROWS, 2 * F], BF16)
            fullA = dram.tile([NA_ROWS, 2 * F], BF16, addr_space="Shared")
            fullB = dram.tile([NB_ROWS, 2 * F], BF16, addr_space="Shared")
            pool_in = dram.tile([F, G], F32)
            pool_out = dram.tile([F, G], F32, addr_space="Shared")

            pool_ps = psP.tile([F, G], F32, tag="pool")

            def epilogue(b, aggT, W_sb, brow_sb, layer):
                agg_sb = ep.tile([F, P], BF16, tag="agg_sb")
                nc.scalar.copy(agg_sb[:], aggT[:])
                h_ps = psH.tile([P, F], F32, tag="h")
                nc.tensor.matmul(h_ps[:], lhsT=agg_sb[:], rhs=W_sb[:],
                                 start=True, stop=False)
                nc.tensor.matmul(h_ps[:], lhsT=ones_sb[:], rhs=brow_sb[:],
                                 start=False, stop=True)
                if layer == 1:
                    nc.scalar.activation(own_sb[:, b, :], h_ps[:],
                                         mybir.ActivationFunctionType.Tanh)
                    if b < ABLOCKS:
                        r0 = b * P
                        nc.sync.dma_start(shardA[r0:r0 + P, 0:F],
                                          own_sb[:, b, :])
                    else:
                        r0 = (b - ABLOCKS) * P
                        rows = min(P, SB_ROWS - r0)
                        nc.sync.dma_start(shardB[r0:r0 + rows, 0:F],
                                          own_sb[:rows, b, :])
                else:
                    h2t = ep.tile([P, F], BF16, tag="h2t")
                    nc.scalar.activation(h2t[:], h_ps[:],
                                         mybir.ActivationFunctionType.Tanh)
                    nc.tensor.matmul(pool_ps[:], lhsT=h2t[:],
                                     rhs=Sp_sb[:, b, :],
                                     start=(b == 0), stop=(b == NB - 1),
                                     skip_group_check=True)

            # =================== Layer 1 (streamed) ===================
            for s in range(NSET):
                ns = int(n_set[s])
                c0 = int(set_off[s])
                st = stp.tile([P, ns * F], BF16, tag="st")
                nc.scalar.dma_start(st[:], xs_in[:, c0 * F:(c0 + ns) * F])
                S_t = smp.tile([P, ns * P], BF16, tag="Sm")
                nc.sync.dma_start(S_t[:], Sm_in[:, c0 * P:(c0 + ns) * P])
                mms = sched[s]
                for b in _set_blocks(s):
                    kis = [k for k, (bb, kind, col) in enumerate(mms) if bb == b]
                    aggT = psA.tile([F, P], F32, tag="aggT")
                    for j, k in enumerate(kis):
                        nc.tensor.matmul(
                            aggT[:],
                            lhsT=st[:, k * F:(k + 1) * F],
                            rhs=S_t[:, k * P:(k + 1) * P],
                            start=(j == 0), stop=(j == len(kis) - 1),
                        )
                    epilogue(b, aggT, W1_sb, b1_sb, 1)
                    if b == ABLOCKS - 1:
                        nc.gpsimd.collective_compute(
                            "AllGather", mybir.AluOpType.bypass,
                            ins=[shardA.opt()], outs=[fullA.opt()],
                            replica_groups=[list(range(C))],
                        )

            # ============ Layer 2, A half (after L1; AG-B hides inside) ===
            l1done = ep.tile([P, F], BF16, tag="l1done")
            nc.gpsimd.tensor_copy(l1done[:], own_sb[:, NB - 1, :])
            acol = 0
            for s in range(NSET):
                nAs = int(nA_set[s])
                c0 = int(set_off[s])
                gtA = gp.tile([P, nAs, 2 * F], BF16, tag="gtA")
                for (p0, p1) in _pieces(nAs, 2):
                    q = gq[0] % 4
                    gq[0] += 1
                    nc.gpsimd.dma_gather(
                        gtA[:, p0:p1, :], fullA[:],
                        idxA_sb[:, acol + p0 * 8:acol + p1 * 8],
                        (p1 - p0) * P, (p1 - p0) * P, 2 * F,
                        single_packet=False, queue_num=q,
                    )
                SA_t = smA.tile([P, nAs * P], BF16, tag="SA")
                ao = 0
                for b in _set_blocks(s):
                    scol, a0, b0 = runs[s][b]
                    na = int(nchA[b])
                    nc.sync.dma_start(
                        SA_t[:, ao * P:(ao + na) * P],
                        Sm_in[:, (c0 + a0) * P:(c0 + a0 + na) * P])
                    aggT = psA.tile([F, P], F32, tag="aggT")
                    for j in range(na):
                        nc.tensor.matmul(
                            aggT[:], lhsT=gtA[:, ao + j, 0:F],
                            rhs=SA_t[:, (ao + j) * P:(ao + j + 1) * P],
                            start=(j == 0), stop=(j == na - 1),
                        )
                    nc.scalar.copy(aggA_sb[:, b, :], aggT[:])
                    ao += na
                acol += nAs * 8
                if s == 6:
                    nc.gpsimd.collective_compute(
                        "AllGather", mybir.AluOpType.bypass,
                        ins=[shardB.opt()], outs=[fullB.opt()],
                        replica_groups=[list(range(C))],
                    )

            # =================== Layer 2, B half ===================
            bcol = 0
            for s in range(NSET):
                nBs = int(nB_set[s])
                c0 = int(set_off[s])
                gtB = gp.tile([P, nBs, 2 * F], BF16, tag="gtB")
                for (p0, p1) in _pieces(nBs, 2):
                    q = gq[0] % 4
                    gq[0] += 1
                    nc.gpsimd.dma_gather(
                        gtB[:, p0:p1, :], fullB[:],
                        idxB_sb[:, bcol + p0 * 8:bcol + p1 * 8],
                        (p1 - p0) * P, (p1 - p0) * P, 2 * F,
                        single_packet=False, queue_num=q,
                    )
                SB_t = smB.tile([P, (nBs + SBLK) * P], BF16, tag="SB")
                bo = 0
                srow = 0
                srows = {}
                for b in _set_blocks(s):
                    scol, a0, b0 = runs[s][b]
                    nb = int(nchB[b])
                    # self col + B run
                    nc.sync.dma_start(
                        SB_t[:, srow * P:(srow + 1) * P],
                        Sm_in[:, (c0 + scol) * P:(c0 + scol + 1) * P])
                    nc.sync.dma_start(
                        SB_t[:, (srow + 1) * P:(srow + 1 + nb) * P],
                        Sm_in[:, (c0 + b0) * P:(c0 + b0 + nb) * P])
                    srows[b] = srow
                    srow += 1 + nb
                for b in _set_blocks(s):
                    nb = int(nchB[b])
                    sr = srows[b]
                    aggT = psA.tile([F, P], F32, tag="aggT")
                    nc.tensor.matmul(aggT[:], lhsT=id64_sb[:],
                                     rhs=aggA_sb[:, b, :],
                                     start=True, stop=False)
                    nc.tensor.matmul(aggT[:], lhsT=own_sb[:, b, :],
                                     rhs=SB_t[:, sr * P:(sr + 1) * P],
                                     start=False, stop=False)
                    for j in range(nb):
                        nc.tensor.matmul(
                            aggT[:], lhsT=gtB[:, bo + j, 0:F],
                            rhs=SB_t[:, (sr + 1 + j) * P:(sr + 2 + j) * P],
                            start=False, stop=(j == nb - 1),
                        )
                    epilogue(b, aggT, W2_sb, b2_sb, 2)
                    bo += nb
                bcol += nBs * 8

            # ---- pooled tail ----
            poolT = ep.tile([F, G], F32, tag="poolT")
            nc.vector.tensor_copy(poolT[:], pool_ps[:])
            nc.sync.dma_start(pool_in[:], poolT[:])
            nc.gpsimd.collective_compute(
                "AllReduce", mybir.AluOpType.add,
                ins=[pool_in.opt()], outs=[pool_out.opt()],
                replica_groups=[list(range(C))],
            )
            poolR = ep.tile([F, G], F32, tag="poolR")
            nc.sync.dma_start(poolR[:], pool_out[:])
            nc.vector.tensor_mul(poolR[:], poolR[:], invc_sb[:])
            fc_ps = psP.tile([G, OUT], F32, tag="fc")
            nc.tensor.matmul(fc_ps[:], lhsT=poolR[:], rhs=Wfc_sb[:],
                             start=True, stop=True)
            out_sb = ep.tile([G, OUT], F32, tag="out_sb")
            nc.vector.tensor_add(out_sb[:], fc_ps[:], bfc_sb[:])
            nc.sync.dma_start(out[:], out_sb[:])

    nc.compile()
    return nc


def _in_maps(plan, per_core, invc, W1, b1, W2, b2, Wfc, bfc):
    com = dict(
        id64=np.eye(F, dtype=np.float32).astype(BF),
        W1=np.asarray(W1, np.float32).astype(BF),
        b1r=np.asarray(b1, np.float32).reshape(1, F).astype(BF),
        W2=np.asarray(W2, np.float32).astype(BF),
        b2r=np.asarray(b2, np.float32).reshape(1, F).astype(BF),
        Wfc=np.ascontiguousarray(np.asarray(Wfc, np.float32)),
        bfcb=np.tile(np.asarray(bfc, np.float32), (P, 1)),
        invc=np.tile(invc, (F, 1)),
    )
    maps = []
    for c in range(C):
        m = dict(com)
        m.update(per_core[c])
        maps.append({k: np.ascontiguousarray(v) for k, v in m.items()})
    return maps


_RUN_KWARGS = {}


def kernel(x, src, dst, batch, W1, b1, W2, b2, Wfc, bfc):
    plan, per_core, invc = _preprocess(x, src, dst, batch)
    nc = _build(plan)
    maps = _in_maps(plan, per_core, invc, W1, b1, W2, b2, Wfc, bfc)
    res = bass_utils.run_bass_kernel_spmd(
        nc, maps, core_ids=list(range(C)), **_RUN_KWARGS
    )
    kernel.last_results = res
    return np.asarray(res.results[0]["out"], np.float32)
